# revision 2
# baseline (speedup 1.0000x reference)
"""Trainium2 Bass kernel for nn_DigitConvolutionalModel (dense_cnn).

Model: y = relu(conv3x3(x) @ w1.T + b1) @ w2.T + b2, x: [65536, 784] f32.

Strategy:
  * The 3x3 valid conv (784 -> 676) and FC1 (676 -> 128) are both linear,
    so they fuse on the host into one effective weight W1e = w1 @ C with
    shape [128, 784] (C is the sparse conv operator). The device then runs
    a pure GEMM pipeline: y = relu(x @ W1e.T + b1) @ w2.T + b2.
  * Pure data parallel over 8 NeuronCores: each core gets 8192 rows of x.
    No collectives; each core produces its own output shard.
  * Matmul operands travel as fp16: tf32-class accuracy for this model's
    value ranges, 1 cycle/row on the PE, half the HBM traffic for x. All
    accumulation stays fp32 in PSUM.
  * The K=784 contraction splits as 7 uniform chunks of 112 (not
    6x128+16): every x byte then streams through one dense 112-partition
    DMA pipe, there is no slow 16-partition tail load on the critical
    path, and the PE pass count (7) is unchanged since matmul cost is
    N cycles regardless of K.
  * Per-core x is pre-tiled on the host into one contiguous DRAM region
    per load ([112, 7, ncols]) so each load is 112 descriptors x 14 KB.
    Loads taper 512,512,1024x6,512,512 columns: quick first block, high
    mid-stream efficiency (~430 GB/s measured), short pipeline drain.
    All loads are SBUF-resident (no slot recycling -> the x DMA stream
    never waits on compute).
  * Weights (one packed fp16 tensor) + biases go on the scalar HWDGE
    ring so they land in parallel with the first x loads on the sync
    ring; output stores also issue from the scalar ring.
  * Per 512-column block: one PSUM bank accumulates 7 FC1 matmuls,
    fused bias+ReLU on the vector engine (PSUM -> SBUF fp16), one
    [10, 512] FC2 matmul, FC2 bias on the scalar engine, HWDGE store.
    Output returns as yT [10, 8192] per core; the host transposes.
  * The PE HAM clock gate defaults to 1.2 GHz and needs ~3.4us of
    sustained activity to ramp to 2.4 GHz. Eight dummy matmuls bridge
    the window between engine start (~8us: event-semaphore preamble +
    first DMA latency) and the first x block landing (~11.5us), so all
    real matmuls run at full clock and compute keeps pace with the DMA
    stream instead of trailing it by ~19us.
  * Tile/walrus quirks handled explicitly: this walrus allows ONE sync
    wait per instruction, so multi-waits are split into event-semaphore
    chains (bass_rust.generate_event_semaphores) and tiny dummy bf16
    ldweights "probes" absorb cross-engine waits into the PE stream
    ahead of each matmul group.
"""

import numpy as np

import concourse.bass as bass
import concourse.mybir as mybir
import concourse.tile as tile
from concourse.bass import ts
from concourse.bass_utils import run_bass_kernel_spmd

H = W = 28
KH = KW = 3
CIN = H * W  # 784
HID = 128
OUT = 10
B_TOTAL = 65536
NCORES = 8
BS = B_TOTAL // NCORES  # 8192 rows per core
NB = 512  # batch columns per psum block (fp32 PSUM bank limit)
# uniform contraction split: 7 chunks of 112 (7 * 112 = 784, no tail)
KCH = 112
KC = 7
NWARM = 8  # HAM warm-up dummy matmuls

# x load schedule (column start, width): taper both ends
LOADS = (
    [(0, NB), (NB, NB)]
    + [(cs, 1024) for cs in range(1024, BS - 1024, 1024)]
    + [(BS - 1024, NB), (BS - NB, NB)]
)
assert sum(n for _, n in LOADS) == BS

HOST_DT = np.float16


def _build_nc():
    f32 = mybir.dt.float32
    mdt = mybir.dt.float16
    nc = bass.Bass()
    # one contiguous DRAM tensor per x load: [112, 7, ncols]
    xts = [
        nc.dram_tensor(f"x{li}", [KCH, KC, ncols], mdt, kind="ExternalInput")
        for li, (_, ncols) in enumerate(LOADS)
    ]
    # all fp16 weights packed into one tensor -> one DMA:
    # rows 0:112 of cols 0:896 = w1e chunks [k, c, m], cols 896:906 = w2t
    wpk = nc.dram_tensor("wpk", [HID, 906], mdt, kind="ExternalInput")
    # both biases in one f32 tensor: col 0 = b1, col 1 rows 0:10 = b2
    bd = nc.dram_tensor("bd", [HID, 2], f32, kind="ExternalInput")
    yt = nc.dram_tensor("yt", [OUT, BS], f32, kind="ExternalOutput")

    with tile.TileContext(nc) as tc:
        with (
            tc.tile_pool(name="consts", bufs=1) as consts,
            tc.tile_pool(name="xin", bufs=1) as xin,
            tc.tile_pool(name="hpool", bufs=8) as hpool,
            tc.tile_pool(name="opool", bufs=6) as opool,
            tc.tile_pool(name="ps1", bufs=4, space="PSUM") as ps1p,
            tc.tile_pool(name="ps2", bufs=2, space="PSUM") as ps2p,
        ):
            # x loads: all SBUF-resident, one tag per load, issued on the
            # sync HWDGE ring in stream order. No recycling dependencies.
            x_tiles = []
            for li, (_, ncols) in enumerate(LOADS):
                x_t = xin.tile([KCH, KC, ncols], mdt, tag=f"x{li}", name=f"x{li}")
                nc.sync.dma_start(x_t[:], xts[li][:])
                x_tiles.append(x_t)

            # weights + biases on the scalar HWDGE ring (parallel with x)
            wpk_t = consts.tile([HID, 906], mdt)
            nc.scalar.dma_start(wpk_t[:], wpk[:])
            w1_t = wpk_t[0:KCH, 0:896].rearrange("k (c m) -> k c m", c=KC)
            w2_t = wpk_t[:, 896:906]
            bd_t = consts.tile([HID, 2], f32)
            nc.scalar.dma_start(bd_t[:], bd[:])
            b1_t = bd_t[:, 0:1]
            b2_t = bd_t[0:OUT, 1:2]

            # Pre-touch the bias tiles on their consumer engines (b1 on DVE,
            # b2 on ACT) so the relu / bias-add instructions don't need a
            # second sync-wait for the bias DMA (walrus: 1 wait per inst).
            b1_probe = consts.tile([1, 1], f32)
            nc.vector.tensor_copy(b1_probe[:], b1_t[0:1, 0:1])
            b2_probe = consts.tile([1, 1], f32)
            nc.scalar.copy(b2_probe[:], b2_t[0:1, 0:1])

            # Matmuls self-load their weights, so every semaphore wait lands
            # on the Matmult itself — and walrus only allows one sync-wait
            # there. Tiny dummy bf16 ldweights "probes" reading 1 element of
            # a tile absorb the cross-engine waits into the PE's in-order
            # stream before each matmul group. The loaded garbage weight is
            # irrelevant (the real matmuls self-load).
            def probe(ap):
                nc.tensor.ldweights(ap[0:1, 0:1].bitcast(mybir.dt.bfloat16))

            probe(w1_t[:, 0, :])
            probe(w2_t[:])

            # HAM warm-up: dummy matmuls over a zeroed scratch tile bridge
            # the PE-idle window until the first x block lands, so the
            # clock gate is at 2.4 GHz for every real matmul.
            scratch = consts.tile([HID, NB], mdt)
            nc.gpsimd.memset(scratch[:], 0.0)
            psd = ps1p.tile([HID, NB], f32, tag="ps")
            for _ in range(NWARM):
                nc.tensor.matmul(
                    psd[:], scratch[:, 0:HID], scratch[:], start=True, stop=True
                )

            # block bi (columns [bi*NB, bi*NB+NB)) -> (load idx, col offset)
            def block_view(bi):
                cs = bi * NB
                for li, (ls, ncols) in enumerate(LOADS):
                    if ls <= cs < ls + ncols:
                        return x_tiles[li], cs - ls
                raise AssertionError

            NBLK = BS // NB  # 16
            for bi in range(NBLK):
                x_t, off = block_view(bi)
                probe(x_t[:, 0, off : off + 1])
                ps_si = ps1p.tile([HID, NB], f32, tag="ps")
                for c in range(KC):
                    nc.tensor.matmul(
                        ps_si[:],
                        w1_t[:, c, :],
                        x_t[:, c, off : off + NB],
                        start=(c == 0),
                        stop=(c == KC - 1),
                    )
                # relu+bias on DVE: h = max(ps + b1, 0), PSUM -> SBUF fp16
                h = hpool.tile([HID, NB], mdt, tag="h")
                nc.vector.tensor_scalar(
                    h[:],
                    ps_si[:],
                    b1_t[:],
                    0.0,
                    mybir.AluOpType.add,
                    mybir.AluOpType.max,
                )
                probe(h[:])
                ps2 = ps2p.tile([OUT, NB], f32, tag="ps2")
                nc.tensor.matmul(ps2[:], w2_t[:], h[:], start=True, stop=True)
                # FC2 bias on the (otherwise idle) scalar engine, then a
                # HWDGE store from the same sequencer
                o = opool.tile([OUT, NB], f32, tag="o")
                nc.scalar.activation(
                    o[:],
                    ps2[:],
                    mybir.ActivationFunctionType.Identity,
                    bias=b2_t[:],
                )
                nc.scalar.dma_start(yt[:, ts(bi, NB)], o[:])

    # This walrus build allows one sync-wait per instruction; Tile emits
    # multi-waits (e.g. slot-recycle WAW + readers-release on DMAs). Split
    # them into event-semaphore chains, same as bacc.compile() does.
    import bass_rust

    bass_rust.generate_event_semaphores(nc)
    return nc


def _fuse_conv_fc1(conv_w, w1):
    """W1e = w1 @ C where C is the 3x3 valid-conv operator [676, 784]."""
    cw = np.asarray(conv_w, np.float64).reshape(KH, KW)
    w1_r = np.asarray(w1, np.float64).reshape(HID, H - KH + 1, W - KW + 1)
    w1e = np.zeros((HID, H, W), np.float64)
    for a in range(KH):
        for b in range(KW):
            w1e[:, a : a + H - KH + 1, b : b + W - KW + 1] += w1_r * cw[a, b]
    return w1e.reshape(HID, CIN).astype(np.float32)


def _core_x(x_shard):
    """Pre-tile one core's x rows [BS, 784] into per-load device tensors
    x{li} [112, 7, ncols] (feature f = c*112 + k on partition k)."""
    out = {}
    for li, (cs, ncols) in enumerate(LOADS):
        blk = x_shard[cs : cs + ncols, :].reshape(ncols, KC, KCH)
        out[f"x{li}"] = np.ascontiguousarray(
            blk.transpose(2, 1, 0).astype(HOST_DT)
        )
    return out


def _host_weights(conv_w, w1, b1, w2, b2):
    """Pack all fp16 weights into wpk [128, 906] and biases into bd."""
    w1e_t = _fuse_conv_fc1(conv_w, w1).T.astype(HOST_DT)  # [784, 128]
    w2t = np.asarray(w2, np.float32).T.astype(HOST_DT)  # [128, 10]
    wpk = np.zeros((HID, 906), HOST_DT)
    wpk[0:KCH, 0:896] = (
        w1e_t.reshape(KC, KCH, HID).transpose(1, 0, 2).reshape(KCH, -1)
    )
    wpk[:, 896:906] = w2t
    bd = np.zeros((HID, 2), np.float32)
    bd[:, 0] = np.asarray(b1, np.float32)
    bd[0:OUT, 1] = np.asarray(b2, np.float32)
    return np.ascontiguousarray(wpk), np.ascontiguousarray(bd)


def _run(x, conv_w, w1, b1, w2, b2, trace=False):
    x = np.asarray(x, np.float32)
    wpk, bd = _host_weights(conv_w, w1, b1, w2, b2)

    nc = _build_nc()
    in_maps = []
    for c in range(NCORES):
        m = _core_x(x[c * BS : (c + 1) * BS])
        m["wpk"] = wpk
        m["bd"] = bd
        in_maps.append(m)
    res = run_bass_kernel_spmd(nc, in_maps, list(range(NCORES)), trace=trace)

    y = np.empty((B_TOTAL, OUT), np.float32)
    for c, r in enumerate(res.results):
        y[c * BS : (c + 1) * BS] = r["yt"].T
    return y, res


def kernel(x, conv_w, w1, b1, w2, b2):
    y, _ = _run(x, conv_w, w1, b1, w2, b2)
    return y


# revision 3
# speedup vs baseline: 1.1244x; 1.1244x over previous
"""Trainium2 Bass kernel for nn_DigitConvolutionalModel (dense_cnn).

Model: y = relu(conv3x3(x) @ w1.T + b1) @ w2.T + b2, x: [65536, 784] f32.

Strategy:
  * The 3x3 valid conv (784 -> 676) and FC1 (676 -> 128) are both linear,
    so they fuse on the host into one effective weight W1e = w1 @ C with
    shape [128, 784] (C is the sparse conv operator). The device then runs
    a pure GEMM pipeline: y = relu(x @ W1e.T + b1) @ w2.T + b2.
  * Pure data parallel over 8 NeuronCores: each core gets 8192 rows of x.
    No collectives; each core produces its own output shard.
  * Matmul operands travel as fp16: tf32-class accuracy for this model's
    value ranges, 1 cycle/row on the PE, half the HBM traffic for x. All
    accumulation stays fp32 in PSUM.
  * The K=784 contraction splits 6x128 + 16. The bulk x loads use
    exactly 128 partitions -> one 12 KB descriptor per partition, and
    the HWDGE balancer assigns descriptors port-matched to the 16 SDMA
    engines (~25 GB/s/engine, ~405 GB/s aggregate). Measured: any other
    partition count (16-partition tail strip, 112-partition uniform
    split) breaks the port match and drops to ~16 GB/s/engine.
  * The 16-feature tail for the whole batch travels in ONE [128, 1024]
    tile: block bi's tail sits on partition strip 16*(bi%8) covering
    columns 512*(bi//8). Per block, a standard K=128 matmul with a
    block-sparse weight (w1 tail rows placed on the matching 16-row
    strip, zero elsewhere) accumulates the tail into the same PSUM bank
    as the 6 main chunks -- full-width DMA, no extra PE passes, no
    tile_position tricks.
  * Per-core x is pre-tiled on the host into one contiguous DRAM region
    per load ([128, 6, ncols]) so every load is 128 descriptors x
    6*ncols*2 B. Loads taper 512,512,1024x6,512,512 columns: quick first
    block, high mid-stream efficiency, short pipeline drain. All loads
    are SBUF-resident (no slot recycling -> the x DMA stream never waits
    on compute). Weights/biases/tail go on the scalar HWDGE ring so they
    land in parallel with the first x loads on the sync ring.
  * Per 512-column block: one PSUM bank accumulates 6+1 FC1 matmuls,
    fused bias+ReLU on the vector engine (PSUM -> SBUF fp16), one
    [10, 512] FC2 matmul, FC2 bias on the scalar engine, HWDGE store.
    The FC2 matmul of block i issues AFTER block i+1's FC1 group so the
    in-order PE never stalls on the DVE relu latency. Output returns as
    yT [10, 8192] per core; the host transposes.
  * The PE HAM clock gate defaults to 1.2 GHz and needs ~3.4us of
    sustained activity to ramp to 2.4 GHz. Eight dummy matmuls bridge
    the window between engine start (~8us: event-semaphore preamble +
    first DMA latency) and the first x block landing, so real matmuls
    run at full clock and compute keeps pace with the DMA stream.
  * Tile/walrus quirks handled explicitly: this walrus allows ONE sync
    wait per instruction, so multi-waits are split into event-semaphore
    chains (bass_rust.generate_event_semaphores) and tiny dummy bf16
    ldweights "probes" absorb cross-engine waits into the PE stream
    ahead of each matmul group.
"""

import numpy as np

import concourse.bass as bass
import concourse.mybir as mybir
import concourse.tile as tile
from concourse.bass import ts
from concourse.bass_utils import run_bass_kernel_spmd

H = W = 28
KH = KW = 3
CIN = H * W  # 784
HID = 128
OUT = 10
B_TOTAL = 65536
NCORES = 8
BS = B_TOTAL // NCORES  # 8192 rows per core
NB = 512  # batch columns per psum block (fp32 PSUM bank limit)
NBLK = BS // NB  # 16
KCH = 128
KC = 6  # full chunks (6 * 128 = 768)
KTAIL = CIN - KC * KCH  # 16
NGRP = 8  # tail partition strips (16 rows each)
NWARM = 8  # HAM warm-up dummy matmuls

# x load schedule (column start, width): taper both ends
LOADS = (
    [(0, NB), (NB, NB)]
    + [(cs, 1024) for cs in range(1024, BS - 1024, 1024)]
    + [(BS - 1024, NB), (BS - NB, NB)]
)
assert sum(n for _, n in LOADS) == BS

# packed weight tensor layout (fp16):
#   cols 0:768          w1e main chunks [k, c, m] (c-major)
#   cols 768:1792       8 block-sparse tail variants [k, g, m]
#   cols 1792:1802      w2t
W1OFF, TAILOFF, W2OFF, WCOLS = 0, 768, 1792, 1802

HOST_DT = np.float16


def _build_nc():
    f32 = mybir.dt.float32
    mdt = mybir.dt.float16
    nc = bass.Bass()
    xts = [
        nc.dram_tensor(f"x{li}", [KCH, KC, ncols], mdt, kind="ExternalInput")
        for li, (_, ncols) in enumerate(LOADS)
    ]
    # whole-batch 16-feature tail, packed 8 strips x 512 cols
    xtl = nc.dram_tensor("xtl", [KCH, (NBLK // NGRP) * NB], mdt, kind="ExternalInput")
    wpk = nc.dram_tensor("wpk", [KCH, WCOLS], mdt, kind="ExternalInput")
    # both biases in one f32 tensor: col 0 = b1, col 1 rows 0:10 = b2
    bd = nc.dram_tensor("bd", [HID, 2], f32, kind="ExternalInput")
    yt = nc.dram_tensor("yt", [OUT, BS], f32, kind="ExternalOutput")

    with tile.TileContext(nc) as tc:
        with (
            tc.tile_pool(name="consts", bufs=1) as consts,
            tc.tile_pool(name="xin", bufs=1) as xin,
            tc.tile_pool(name="hpool", bufs=8) as hpool,
            tc.tile_pool(name="opool", bufs=6) as opool,
            tc.tile_pool(name="ps1", bufs=4, space="PSUM") as ps1p,
            tc.tile_pool(name="ps2", bufs=2, space="PSUM") as ps2p,
        ):
            # x loads: all SBUF-resident, one tag per load, issued on the
            # sync HWDGE ring in stream order. No recycling dependencies.
            x_tiles = []
            for li, (_, ncols) in enumerate(LOADS):
                x_t = xin.tile([KCH, KC, ncols], mdt, tag=f"x{li}", name=f"x{li}")
                nc.sync.dma_start(x_t[:], xts[li][:])
                x_tiles.append(x_t)

            # weights + biases + tail on the scalar HWDGE ring
            wpk_t = consts.tile([KCH, WCOLS], mdt)
            nc.scalar.dma_start(wpk_t[:], wpk[:])
            w1_t = wpk_t[:, W1OFF : W1OFF + KC * HID].rearrange(
                "k (c m) -> k c m", c=KC
            )
            wtl_t = wpk_t[:, TAILOFF : TAILOFF + NGRP * HID].rearrange(
                "k (g m) -> k g m", g=NGRP
            )
            w2_t = wpk_t[:, W2OFF : W2OFF + OUT]
            xtl_t = consts.tile([KCH, (NBLK // NGRP) * NB], mdt)
            nc.scalar.dma_start(xtl_t[:], xtl[:])
            bd_t = consts.tile([HID, 2], f32)
            nc.scalar.dma_start(bd_t[:], bd[:])
            b1_t = bd_t[:, 0:1]
            b2_t = bd_t[0:OUT, 1:2]

            # Pre-touch the bias tiles on their consumer engines (b1 on DVE,
            # b2 on ACT) so the relu / bias-add instructions don't need a
            # second sync-wait for the bias DMA (walrus: 1 wait per inst).
            b1_probe = consts.tile([1, 1], f32)
            nc.vector.tensor_copy(b1_probe[:], b1_t[0:1, 0:1])
            b2_probe = consts.tile([1, 1], f32)
            nc.scalar.copy(b2_probe[:], b2_t[0:1, 0:1])

            # Matmuls self-load their weights, so every semaphore wait lands
            # on the Matmult itself -- and walrus only allows one sync-wait
            # there. Tiny dummy bf16 ldweights "probes" reading 1 element of
            # a tile absorb the cross-engine waits into the PE's in-order
            # stream before each matmul group. The loaded garbage weight is
            # irrelevant (the real matmuls self-load).
            def probe(ap):
                nc.tensor.ldweights(ap[0:1, 0:1].bitcast(mybir.dt.bfloat16))

            probe(w1_t[:, 0, :])
            probe(xtl_t[:])
            probe(w2_t[:])

            # HAM warm-up: dummy matmuls over a zeroed scratch tile bridge
            # the PE-idle window until the first x block lands, so the
            # clock gate is at 2.4 GHz for every real matmul.
            scratch = consts.tile([HID, NB], mdt)
            nc.gpsimd.memset(scratch[:], 0.0)
            psd = ps1p.tile([HID, NB], f32, tag="ps")
            for _ in range(NWARM):
                nc.tensor.matmul(
                    psd[:], scratch[:, 0:HID], scratch[:], start=True, stop=True
                )

            # block bi (columns [bi*NB, bi*NB+NB)) -> (load idx, col offset)
            def block_view(bi):
                cs = bi * NB
                for li, (ls, ncols) in enumerate(LOADS):
                    if ls <= cs < ls + ncols:
                        return x_tiles[li], cs - ls
                raise AssertionError

            def fc1(bi):
                x_t, off = block_view(bi)
                probe(x_t[:, 0, off : off + 1])
                ps_si = ps1p.tile([HID, NB], f32, tag="ps", name=f"ps{bi}")
                for c in range(KC):
                    nc.tensor.matmul(
                        ps_si[:],
                        w1_t[:, c, :],
                        x_t[:, c, off : off + NB],
                        start=(c == 0),
                        stop=False,
                    )
                # 16-feature tail: block-sparse K=128 matmul from the strip
                nc.tensor.matmul(
                    ps_si[:],
                    wtl_t[:, bi % NGRP, :],
                    xtl_t[:, ts(bi // NGRP, NB)],
                    start=False,
                    stop=True,
                )
                return ps_si

            def fc2(bi, ps_si):
                # relu+bias on DVE: h = max(ps + b1, 0), PSUM -> SBUF fp16
                h = hpool.tile([HID, NB], mdt, tag="h", name=f"h{bi}")
                nc.vector.tensor_scalar(
                    h[:],
                    ps_si[:],
                    b1_t[:],
                    0.0,
                    mybir.AluOpType.add,
                    mybir.AluOpType.max,
                )
                probe(h[:])
                ps2 = ps2p.tile([OUT, NB], f32, tag="ps2", name=f"o{bi}")
                nc.tensor.matmul(ps2[:], w2_t[:], h[:], start=True, stop=True)
                # FC2 bias on the (otherwise idle) scalar engine, then a
                # HWDGE store from the same sequencer
                o = opool.tile([OUT, NB], f32, tag="o", name=f"y{bi}")
                nc.scalar.activation(
                    o[:],
                    ps2[:],
                    mybir.ActivationFunctionType.Identity,
                    bias=b2_t[:],
                )
                nc.scalar.dma_start(yt[:, ts(bi, NB)], o[:])

            # software pipeline: FC2 of block i issues after FC1 of block
            # i+1, so the PE never waits on the DVE relu.
            pending = None
            for bi in range(NBLK):
                ps_si = fc1(bi)
                if pending is not None:
                    fc2(*pending)
                pending = (bi, ps_si)
            fc2(*pending)

    # This walrus build allows one sync-wait per instruction; Tile emits
    # multi-waits (e.g. slot-recycle WAW + readers-release on DMAs). Split
    # them into event-semaphore chains, same as bacc.compile() does.
    import bass_rust

    bass_rust.generate_event_semaphores(nc)
    return nc


def _fuse_conv_fc1(conv_w, w1):
    """W1e = w1 @ C where C is the 3x3 valid-conv operator [676, 784]."""
    cw = np.asarray(conv_w, np.float64).reshape(KH, KW)
    w1_r = np.asarray(w1, np.float64).reshape(HID, H - KH + 1, W - KW + 1)
    w1e = np.zeros((HID, H, W), np.float64)
    for a in range(KH):
        for b in range(KW):
            w1e[:, a : a + H - KH + 1, b : b + W - KW + 1] += w1_r * cw[a, b]
    return w1e.reshape(HID, CIN).astype(np.float32)


def _core_x(x_shard):
    """Pre-tile one core's x rows [BS, 784]: per-load tensors x{li}
    [128, 6, ncols] (feature f = c*128 + k on partition k) plus the
    packed tail xtl [128, 1024] (block bi's 16 tail features on
    partition strip 16*(bi%8), columns 512*(bi//8))."""
    out = {}
    for li, (cs, ncols) in enumerate(LOADS):
        blk = x_shard[cs : cs + ncols, : KC * KCH].reshape(ncols, KC, KCH)
        out[f"x{li}"] = np.ascontiguousarray(blk.transpose(2, 1, 0).astype(HOST_DT))
    xtl = np.zeros((KCH, (NBLK // NGRP) * NB), HOST_DT)
    tail = x_shard[:, KC * KCH :].astype(HOST_DT)  # [BS, 16]
    for bi in range(NBLK):
        g, cb = bi % NGRP, bi // NGRP
        xtl[16 * g : 16 * g + KTAIL, cb * NB : (cb + 1) * NB] = tail[
            bi * NB : (bi + 1) * NB
        ].T
    out["xtl"] = np.ascontiguousarray(xtl)
    return out


def _host_weights(conv_w, w1, b1, w2, b2):
    """Pack all fp16 weights into wpk [128, 1802] and biases into bd."""
    w1e_t = _fuse_conv_fc1(conv_w, w1).T.astype(HOST_DT)  # [784, 128]
    w2t = np.asarray(w2, np.float32).T.astype(HOST_DT)  # [128, 10]
    wpk = np.zeros((KCH, WCOLS), HOST_DT)
    wpk[:, W1OFF : W1OFF + KC * HID] = (
        w1e_t[: KC * KCH].reshape(KC, KCH, HID).transpose(1, 0, 2).reshape(KCH, -1)
    )
    # 8 block-sparse tail variants: variant g has the 16 tail rows of
    # w1e on partition strip 16g, zeros elsewhere
    tail_w = w1e_t[KC * KCH :]  # [16, 128]
    for g in range(NGRP):
        wpk[16 * g : 16 * g + KTAIL, TAILOFF + g * HID : TAILOFF + (g + 1) * HID] = (
            tail_w
        )
    wpk[:, W2OFF : W2OFF + OUT] = w2t
    bd = np.zeros((HID, 2), np.float32)
    bd[:, 0] = np.asarray(b1, np.float32)
    bd[0:OUT, 1] = np.asarray(b2, np.float32)
    return np.ascontiguousarray(wpk), np.ascontiguousarray(bd)


def _run(x, conv_w, w1, b1, w2, b2, trace=False):
    x = np.asarray(x, np.float32)
    wpk, bd = _host_weights(conv_w, w1, b1, w2, b2)

    nc = _build_nc()
    in_maps = []
    for c in range(NCORES):
        m = _core_x(x[c * BS : (c + 1) * BS])
        m["wpk"] = wpk
        m["bd"] = bd
        in_maps.append(m)
    res = run_bass_kernel_spmd(nc, in_maps, list(range(NCORES)), trace=trace)

    y = np.empty((B_TOTAL, OUT), np.float32)
    for c, r in enumerate(res.results):
        y[c * BS : (c + 1) * BS] = r["yt"].T
    return y, res


def kernel(x, conv_w, w1, b1, w2, b2):
    y, _ = _run(x, conv_w, w1, b1, w2, b2)
    return y


# revision 10
# speedup vs baseline: 1.2222x; 1.0871x over previous
"""Trainium2 Bass kernel for nn_DigitConvolutionalModel (dense_cnn).

Model: y = relu(conv3x3(x) @ w1.T + b1) @ w2.T + b2, x: [65536, 784] f32.

Strategy:
  * The 3x3 valid conv (784 -> 676) and FC1 (676 -> 128) are both linear,
    so they fuse on the host into one effective weight W1e = w1 @ C with
    shape [128, 784] (C is the sparse conv operator). The device then runs
    a pure GEMM pipeline: y = relu(x @ W1e.T + b1) @ w2.T + b2.
  * Pure data parallel over 8 NeuronCores: each core gets 8192 rows of x.
    No collectives; each core produces its own output shard.
  * Matmul operands travel as fp16: tf32-class accuracy for this model's
    value ranges, 1 cycle/row on the PE, half the HBM traffic for x. All
    accumulation stays fp32 in PSUM.
  * The K=784 contraction splits 6x128 + 16. The bulk x loads use
    exactly 128 partitions -> one 12 KB descriptor per partition, and
    the HWDGE balancer assigns descriptors port-matched to the 16 SDMA
    engines (~25 GB/s/engine, ~405 GB/s aggregate). Measured: any other
    partition count (16-partition tail strip, 112-partition uniform
    split) breaks the port match and drops to ~16 GB/s/engine.
  * The 16-feature tail for the whole batch travels in ONE [128, 1024]
    tile: block bi's tail sits on partition strip 16*(bi%8) covering
    columns 512*(bi//8). Per block, a standard K=128 matmul with a
    block-sparse weight (w1 tail rows placed on the matching 16-row
    strip, zero elsewhere) accumulates the tail into the same PSUM bank
    as the 6 main chunks -- full-width DMA, no extra PE passes, no
    tile_position tricks.
  * Per-core x is pre-tiled on the host into one contiguous DRAM region
    per load ([128, 6, ncols]) so every load is 128 descriptors x
    6*ncols*2 B. Loads taper 512,512,1024x6,512,512 columns: quick first
    block, high mid-stream efficiency, short pipeline drain. All loads
    are SBUF-resident (no slot recycling -> the x DMA stream never waits
    on compute). Weights/biases/tail go on the scalar HWDGE ring so they
    land in parallel with the first x loads on the sync ring.
  * Per 512-column block: one PSUM bank accumulates 6+1 FC1 matmuls,
    fused bias+ReLU on the vector engine (PSUM -> SBUF fp16), one
    [10, 512] FC2 matmul, FC2 bias on the scalar engine, HWDGE store.
    The FC2 matmul of block i issues AFTER block i+1's FC1 group so the
    in-order PE never stalls on the DVE relu latency. Output returns as
    yT [10, 8192] per core; the host transposes.
  * The PE HAM clock gate defaults to 1.2 GHz and needs ~3.4us of
    sustained activity to ramp to 2.4 GHz. Eight dummy matmuls bridge
    the window between engine start (~8us: event-semaphore preamble +
    first DMA latency) and the first x block landing, so real matmuls
    run at full clock and compute keeps pace with the DMA stream.
  * Tile/walrus quirks handled explicitly: this walrus allows ONE sync
    wait per instruction, so multi-waits are split into event-semaphore
    chains (bass_rust.generate_event_semaphores) and tiny dummy bf16
    ldweights "probes" absorb cross-engine waits into the PE stream
    ahead of each matmul group.
"""

import numpy as np

import concourse.bass as bass
import concourse.mybir as mybir
import concourse.tile as tile
from concourse.bass import ts
from concourse.bass_utils import run_bass_kernel_spmd

H = W = 28
KH = KW = 3
CIN = H * W  # 784
HID = 128
OUT = 10
B_TOTAL = 65536
NCORES = 8
BS = B_TOTAL // NCORES  # 8192 rows per core
NB = 512  # batch columns per psum block (fp32 PSUM bank limit)
NBLK = BS // NB  # 16
KCH = 128
KC = 6  # full chunks (6 * 128 = 768)
KTAIL = CIN - KC * KCH  # 16
NGRP = 8  # tail partition strips (16 rows each)
NWARM = 8  # HAM warm-up dummy matmuls

# x load schedule (column start, width): taper both ends. Exactly 8
# loads: with wpk + bd that is 10 HWDGE DMAs over the scheduler's 8
# round-robin DMAHW semaphore lanes, so only two lane reuses exist and
# both pair a late x load with an early weight load (no issue stalls --
# v3 measured x loads 6+ blocked 10-30us on lane predecessors).
LOADS = [
    (0, 512),
    (512, 512),
    (1024, 1024),
    (2048, 1024),
    (3072, 2048),
    (5120, 1536),
    (6656, 1024),
    (7680, 512),
]
assert sum(n for _, n in LOADS) == BS

# packed weight+tail tensor layout (fp16):
#   cols 0:768          w1e main chunks [k, c, m] (c-major)
#   cols 768:1792       8 block-sparse tail variants [k, g, m]
#   cols 1792:1802      w2t
#   cols 1802:2826      per-core packed x tail strips [128, 1024]
W1OFF, TAILOFF, W2OFF, XTLOFF, WCOLS = 0, 768, 1792, 1802, 2826

HOST_DT = np.float16


def _build_nc():
    f32 = mybir.dt.float32
    mdt = mybir.dt.float16
    nc = bass.Bass()
    xts = [
        nc.dram_tensor(f"x{li}", [KCH, KC, ncols], mdt, kind="ExternalInput")
        for li, (_, ncols) in enumerate(LOADS)
    ]
    wpk = nc.dram_tensor("wpk", [KCH, WCOLS], mdt, kind="ExternalInput")
    # both biases in one f32 tensor: col 0 = b1, col 1 rows 0:10 = b2
    bd = nc.dram_tensor("bd", [HID, 2], f32, kind="ExternalInput")
    yt = nc.dram_tensor("yt", [OUT, BS], f32, kind="ExternalOutput")

    with tile.TileContext(nc) as tc:
        with (
            tc.tile_pool(name="consts", bufs=1) as consts,
            tc.tile_pool(name="xin", bufs=1) as xin,
            tc.tile_pool(name="hpool", bufs=8) as hpool,
            tc.tile_pool(name="opool", bufs=6) as opool,
            tc.tile_pool(name="ps1", bufs=4, space="PSUM") as ps1p,
            tc.tile_pool(name="ps2", bufs=2, space="PSUM") as ps2p,
        ):
            # x loads: all SBUF-resident, one tag per load, issued on the
            # sync HWDGE ring in stream order. No recycling dependencies.
            x_tiles = []
            for li, (_, ncols) in enumerate(LOADS):
                x_t = xin.tile([KCH, KC, ncols], mdt, tag=f"x{li}", name=f"x{li}")
                nc.sync.dma_start(x_t[:], xts[li][:])
                x_tiles.append(x_t)

            # weights + biases + packed tail on the scalar HWDGE ring
            wpk_t = consts.tile([KCH, WCOLS], mdt)
            nc.scalar.dma_start(wpk_t[:], wpk[:])
            w1_t = wpk_t[:, W1OFF : W1OFF + KC * HID].rearrange(
                "k (c m) -> k c m", c=KC
            )
            wtl_t = wpk_t[:, TAILOFF : TAILOFF + NGRP * HID].rearrange(
                "k (g m) -> k g m", g=NGRP
            )
            w2_t = wpk_t[:, W2OFF : W2OFF + OUT]
            xtl_t = wpk_t[:, XTLOFF:WCOLS]
            bd_t = consts.tile([HID, 2], f32)
            nc.scalar.dma_start(bd_t[:], bd[:])
            b1_t = bd_t[:, 0:1]
            b2_t = bd_t[0:OUT, 1:2]

            # Pre-touch the bias tiles on their consumer engines (b1 on DVE,
            # b2 on ACT) so the relu / bias-add instructions don't need a
            # second sync-wait for the bias DMA (walrus: 1 wait per inst).
            b1_probe = consts.tile([1, 1], f32)
            nc.vector.tensor_copy(b1_probe[:], b1_t[0:1, 0:1])
            b2_probe = consts.tile([1, 1], f32)
            nc.scalar.copy(b2_probe[:], b2_t[0:1, 0:1])

            # Matmuls self-load their weights, so every semaphore wait lands
            # on the Matmult itself -- and walrus only allows one sync-wait
            # there. Tiny dummy bf16 ldweights "probes" reading 1 element of
            # a tile absorb the cross-engine waits into the PE's in-order
            # stream before each matmul group. The loaded garbage weight is
            # irrelevant (the real matmuls self-load).
            def probe(ap):
                nc.tensor.ldweights(ap[0:1, 0:1].bitcast(mybir.dt.bfloat16))

            probe(w1_t[:, 0, :])
            probe(xtl_t[:])
            probe(w2_t[:])

            # HAM warm-up: dummy matmuls over a zeroed scratch tile bridge
            # the PE-idle window until the first x block lands, so the
            # clock gate is at 2.4 GHz for every real matmul.
            scratch = consts.tile([HID, NB], mdt)
            nc.gpsimd.memset(scratch[:], 0.0)
            psd = ps1p.tile([HID, NB], f32, tag="ps")
            for _ in range(NWARM):
                nc.tensor.matmul(
                    psd[:], scratch[:, 0:HID], scratch[:], start=True, stop=True
                )

            # block bi (columns [bi*NB, bi*NB+NB)) -> (load idx, col offset)
            def block_view(bi):
                cs = bi * NB
                for li, (ls, ncols) in enumerate(LOADS):
                    if ls <= cs < ls + ncols:
                        return x_tiles[li], cs - ls
                raise AssertionError

            def fc1(bi):
                x_t, off = block_view(bi)
                probe(x_t[:, 0, off : off + 1])
                ps_si = ps1p.tile([HID, NB], f32, tag="ps", name=f"ps{bi}")
                for c in range(KC):
                    nc.tensor.matmul(
                        ps_si[:],
                        w1_t[:, c, :],
                        x_t[:, c, off : off + NB],
                        start=(c == 0),
                        stop=False,
                    )
                # 16-feature tail: block-sparse K=128 matmul from the strip
                nc.tensor.matmul(
                    ps_si[:],
                    wtl_t[:, bi % NGRP, :],
                    xtl_t[:, ts(bi // NGRP, NB)],
                    start=False,
                    stop=True,
                )
                return ps_si

            def fc2(bi, ps_si):
                # relu+bias on DVE: h = max(ps + b1, 0), PSUM -> SBUF fp16
                h = hpool.tile([HID, NB], mdt, tag="h", name=f"h{bi}")
                nc.vector.tensor_scalar(
                    h[:],
                    ps_si[:],
                    b1_t[:],
                    0.0,
                    mybir.AluOpType.add,
                    mybir.AluOpType.max,
                )
                probe(h[:])
                ps2 = ps2p.tile([OUT, NB], f32, tag="ps2", name=f"o{bi}")
                nc.tensor.matmul(ps2[:], w2_t[:], h[:], start=True, stop=True)
                # FC2 bias on the (otherwise idle) scalar engine, then a
                # HWDGE store from the same sequencer
                o = opool.tile([OUT, NB], f32, tag="o", name=f"y{bi}")
                nc.scalar.activation(
                    o[:],
                    ps2[:],
                    mybir.ActivationFunctionType.Identity,
                    bias=b2_t[:],
                )
                # store via SWDGE: the gpsimd engine is otherwise idle and
                # its DMASW semaphore lanes are separate from the 8 DMAHW
                # lanes the x stream needs; issuing from ACT (v3) both
                # serialized the tail behind per-store descriptor
                # generation and burned DMAHW lanes.
                nc.gpsimd.dma_start(yt[:, ts(bi, NB)], o[:])

            # software pipeline: FC2 of block i issues after FC1 of block
            # i+1, so the PE never waits on the DVE relu.
            pending = None
            for bi in range(NBLK):
                ps_si = fc1(bi)
                if pending is not None:
                    fc2(*pending)
                pending = (bi, ps_si)
            fc2(*pending)

    # This walrus build allows one sync-wait per instruction; Tile emits
    # multi-waits (e.g. slot-recycle WAW + readers-release on DMAs). Split
    # them into event-semaphore chains, same as bacc.compile() does.
    import bass_rust

    bass_rust.generate_event_semaphores(nc)
    return nc


def _fuse_conv_fc1(conv_w, w1):
    """W1e = w1 @ C where C is the 3x3 valid-conv operator [676, 784]."""
    cw = np.asarray(conv_w, np.float64).reshape(KH, KW)
    w1_r = np.asarray(w1, np.float64).reshape(HID, H - KH + 1, W - KW + 1)
    w1e = np.zeros((HID, H, W), np.float64)
    for a in range(KH):
        for b in range(KW):
            w1e[:, a : a + H - KH + 1, b : b + W - KW + 1] += w1_r * cw[a, b]
    return w1e.reshape(HID, CIN).astype(np.float32)


def _core_x(x_shard):
    """Pre-tile one core's x rows [BS, 784]: per-load tensors x{li}
    [128, 6, ncols] (feature f = c*128 + k on partition k) plus the
    packed tail xtl [128, 1024] (block bi's 16 tail features on
    partition strip 16*(bi%8), columns 512*(bi//8))."""
    out = {}
    for li, (cs, ncols) in enumerate(LOADS):
        blk = x_shard[cs : cs + ncols, : KC * KCH].reshape(ncols, KC, KCH)
        out[f"x{li}"] = np.ascontiguousarray(blk.transpose(2, 1, 0).astype(HOST_DT))
    xtl = np.zeros((KCH, (NBLK // NGRP) * NB), HOST_DT)
    tail = x_shard[:, KC * KCH :].astype(HOST_DT)  # [BS, 16]
    for bi in range(NBLK):
        g, cb = bi % NGRP, bi // NGRP
        xtl[16 * g : 16 * g + KTAIL, cb * NB : (cb + 1) * NB] = tail[
            bi * NB : (bi + 1) * NB
        ].T
    return out, xtl


def _host_weights(conv_w, w1, b1, w2, b2):
    """Pack all fp16 weights into wpk [128, 2826] and biases into bd.
    Cols XTLOFF: are filled per-core with the packed x tail."""
    w1e_t = _fuse_conv_fc1(conv_w, w1).T.astype(HOST_DT)  # [784, 128]
    w2t = np.asarray(w2, np.float32).T.astype(HOST_DT)  # [128, 10]
    wpk = np.zeros((KCH, WCOLS), HOST_DT)
    wpk[:, W1OFF : W1OFF + KC * HID] = (
        w1e_t[: KC * KCH].reshape(KC, KCH, HID).transpose(1, 0, 2).reshape(KCH, -1)
    )
    # 8 block-sparse tail variants: variant g has the 16 tail rows of
    # w1e on partition strip 16g, zeros elsewhere
    tail_w = w1e_t[KC * KCH :]  # [16, 128]
    for g in range(NGRP):
        wpk[16 * g : 16 * g + KTAIL, TAILOFF + g * HID : TAILOFF + (g + 1) * HID] = (
            tail_w
        )
    wpk[:, W2OFF : W2OFF + OUT] = w2t
    bd = np.zeros((HID, 2), np.float32)
    bd[:, 0] = np.asarray(b1, np.float32)
    bd[0:OUT, 1] = np.asarray(b2, np.float32)
    return np.ascontiguousarray(wpk), np.ascontiguousarray(bd)


def _run(x, conv_w, w1, b1, w2, b2, trace=False):
    x = np.asarray(x, np.float32)
    wpk, bd = _host_weights(conv_w, w1, b1, w2, b2)

    nc = _build_nc()
    in_maps = []
    for c in range(NCORES):
        m, xtl = _core_x(x[c * BS : (c + 1) * BS])
        m["wpk"] = np.ascontiguousarray(
            np.concatenate([wpk[:, :XTLOFF], xtl], axis=1)
        )
        m["bd"] = bd
        in_maps.append(m)
    res = run_bass_kernel_spmd(nc, in_maps, list(range(NCORES)), trace=trace)

    y = np.empty((B_TOTAL, OUT), np.float32)
    for c, r in enumerate(res.results):
        y[c * BS : (c + 1) * BS] = r["yt"].T
    return y, res


def kernel(x, conv_w, w1, b1, w2, b2):
    y, _ = _run(x, conv_w, w1, b1, w2, b2)
    return y
